# revision 11
# baseline (speedup 1.0000x reference)
"""Chamfer distance kernel for 8 Trainium2 NeuronCores.

Problem: points1/points2 [16, 4096, 3] fp32 -> scalar sum over batches of
  mean(min_m d2[n,m]) + mean(min_n d2[n,m]).

Strategy:
  - Data parallel over batch: core c handles batches [2c, 2c+1].
  - d2 tiles produced on TensorE as a K=13 bf16 matmul of augmented feature
    vectors (hi/lo split of coordinates + split squared norms) -> ~fp24
    effective precision at bf16 matmul throughput.
  - Per [128, 2048] fp32 PSUM tile: ScalarE converts to bf16 SBUF; VectorE
    does the column-direction min as a running elementwise min, and the
    row-direction min as a bf16 2x-mode pairwise fold chain plus one small
    1x free-axis reduce (tensor_tensor_reduce would fuse this but faults
    on TRN2 hardware via this toolchain).
  - Column accumulators [128, 2048] (min over n within partition-residue
    class) are DMAed to DRAM; the host does the final 128-way min and the
    fp64 means/sum.
"""

import sys

import numpy as np

for _p in ("/opt/trn_rl_repo",):
    if _p not in sys.path:
        sys.path.insert(0, _p)

import ml_dtypes  # noqa: E402
from contextlib import ExitStack  # noqa: E402

import concourse.bass as bass  # noqa: E402
import concourse.bacc as bacc  # noqa: E402
import concourse.tile as tile  # noqa: E402
from concourse import mybir  # noqa: E402

BF16 = ml_dtypes.bfloat16

B, N, M, D = 16, 4096, 4096, 3
NCORES = 8
BPC = B // NCORES  # batches per core
K = 13  # contraction rows of the augmented feature matmul
P = 128  # partitions / n-chunk size
MW = 2048  # m tile width (4 PSUM banks)
NI = N // P  # 32 n-chunks
NJ = M // MW  # 2 m-halves
BIG = 1.0e30  # min-identity sentinel (finite, bf16-representable)

_CACHE: dict = {}


def _build_bass(reps: int = 1) -> "bass.Bass":
    """One SPMD NeuronCore program: full chamfer partials for BPC batches.

    reps > 1 repeats the whole compute (same outputs) for timing-by-differencing.
    """
    nc = bacc.Bacc()
    f1d = nc.declare_dram_parameter("feat1", [BPC, K, N], mybir.dt.bfloat16, isOutput=False)
    f2d = nc.declare_dram_parameter("feat2", [BPC, K, M], mybir.dt.bfloat16, isOutput=False)
    d1o = nc.declare_dram_parameter("d1out", [BPC, P, NI], mybir.dt.float32, isOutput=True)
    c2o = nc.declare_dram_parameter("colout", [BPC, NJ, P, MW], mybir.dt.bfloat16, isOutput=True)

    mn = mybir.AluOpType.min

    with tile.TileContext(nc) as tc, ExitStack() as ctx:
        singles = ctx.enter_context(tc.tile_pool(name="singles", bufs=1))
        psum_pool = ctx.enter_context(tc.tile_pool(name="psum", bufs=2, space="PSUM"))
        sbt_pool = ctx.enter_context(tc.tile_pool(name="sbt", bufs=4))
        scr_pool = ctx.enter_context(tc.tile_pool(name="scr", bufs=2))
        acc_pool = ctx.enter_context(tc.tile_pool(name="acc", bufs=1))

        inf_tile = singles.tile([P, MW], mybir.dt.bfloat16, tag="inf")
        nc.vector.memset(inf_tile, BIG)

        f1sb = {}
        f2sb = {}
        for b in range(BPC):
            f1sb[b] = singles.tile([K, N], mybir.dt.bfloat16, tag=f"f1_{b}", name=f"f1_{b}")
            nc.gpsimd.dma_start(out=f1sb[b], in_=f1d[b])
            f2sb[b] = singles.tile([K, M], mybir.dt.bfloat16, tag=f"f2_{b}", name=f"f2_{b}")
            nc.gpsimd.dma_start(out=f2sb[b], in_=f2d[b])

        for b in range(BPC):
            d1b = acc_pool.tile([P, NI], mybir.dt.float32, tag=f"d1b_{b}")
            colacc = {}
            for j in range(NJ):
                colacc[j] = acc_pool.tile([P, MW], mybir.dt.bfloat16, tag=f"colacc_{b}_{j}", name=f"colacc_{b}_{j}")

            for rep in range(reps):
              for i in range(NI):
                sbts = []
                for j in range(NJ):
                    ps = psum_pool.tile([P, MW], mybir.dt.float32, tag="ps")
                    lhsT = f1sb[b][:, i * P : (i + 1) * P]
                    for q in range(MW // 512):
                        nc.tensor.matmul(
                            ps[:, q * 512 : (q + 1) * 512],
                            lhsT,
                            f2sb[b][:, j * MW + q * 512 : j * MW + (q + 1) * 512],
                            start=True,
                            stop=True,
                        )
                    sbt = sbt_pool.tile([P, MW], mybir.dt.bfloat16, tag="sbt")
                    nc.scalar.copy(out=sbt, in_=ps)
                    sbts.append(sbt)

                    # column-direction min (dist2): running elementwise min.
                    nc.vector.tensor_tensor(
                        out=colacc[j],
                        in0=sbt,
                        in1=(inf_tile if i == 0 and rep == 0 else colacc[j]),
                        op=mn,
                    )

                # row-direction min (dist1): fold the two j-half tiles, then
                # 2x-mode fold chain + one small 1x free-axis reduce.
                # (tensor_tensor_reduce would fuse this but faults on HW.)
                f0 = scr_pool.tile([P, MW], mybir.dt.bfloat16, tag="f0")
                nc.vector.tensor_tensor(out=f0, in0=sbts[0], in1=sbts[1], op=mn)
                f1t = scr_pool.tile([P, MW // 2], mybir.dt.bfloat16, tag="f1t")
                nc.vector.tensor_tensor(
                    out=f1t, in0=f0[:, : MW // 2], in1=f0[:, MW // 2 :], op=mn
                )
                f2t = scr_pool.tile([P, MW // 4], mybir.dt.bfloat16, tag="f2t")
                nc.vector.tensor_tensor(
                    out=f2t, in0=f1t[:, : MW // 4], in1=f1t[:, MW // 4 :], op=mn
                )
                f3t = scr_pool.tile([P, MW // 8], mybir.dt.bfloat16, tag="f3t")
                nc.vector.tensor_tensor(
                    out=f3t, in0=f2t[:, : MW // 8], in1=f2t[:, MW // 8 :], op=mn
                )
                nc.vector.tensor_reduce(
                    out=d1b[:, i : i + 1], in_=f3t, axis=mybir.AxisListType.X, op=mn
                )

            nc.sync.dma_start(out=d1o[b], in_=d1b)
            for j in range(NJ):
                nc.sync.dma_start(out=c2o[b, j], in_=colacc[j])

    return nc


def get_bass(reps: int = 1) -> "bass.Bass":
    key = f"nc{reps}"
    if key not in _CACHE:
        nc = _build_bass(reps)
        nc.finalize()  # runs Bacc passes (reg alloc, matmul wait splitting)
        _CACHE[key] = nc
    return _CACHE[key]


def make_features(points1: np.ndarray, points2: np.ndarray):
    """Augmented bf16 feature rows so that lhsT.T @ rhs == squared distance.

    Per coordinate x (hi/lo bf16 split, x ~= xh + xl):
      x*x' ~= xh*x'h + xh*x'l + xl*x'h   (drops xl*x'l ~ 2^-18 rel)
    Rows (lhs from points1 side, rhs from points2 side):
      9 product rows + [n_hi, n_lo, 1, 1] vs [1, 1, n'_hi, n'_lo]
    where n = ||hi+lo||^2 computed in fp64 and split into bf16 hi/lo.
    Returns (feat1, feat2) with shape [B, K, N] bf16.
    """
    def split(x64):
        hi = x64.astype(BF16)
        lo = (x64 - hi.astype(np.float64)).astype(BF16)
        return hi, lo

    p1 = np.asarray(points1, dtype=np.float64)
    p2 = np.asarray(points2, dtype=np.float64)
    bsz = p1.shape[0]

    def feats(p, side):
        # p: [B, N, 3] fp64
        hi, lo = split(p)  # bf16 [B, N, 3]
        hi64 = hi.astype(np.float64)
        lo64 = lo.astype(np.float64)
        ptilde = hi64 + lo64
        n = (ptilde * ptilde).sum(-1)  # [B, N] fp64, exact-ish
        nhi = n.astype(BF16)
        nlo = (n - nhi.astype(np.float64)).astype(BF16)
        ones = np.ones(n.shape, dtype=BF16)
        rows = []
        if side == 1:
            for d in range(3):
                rows += [hi[..., d], hi[..., d], lo[..., d]]
            rows += [nhi, nlo, ones, ones]
        else:
            m2hi = (-2.0 * hi64).astype(BF16)  # exact: *2 is a power of two
            m2lo = (-2.0 * lo64).astype(BF16)
            for d in range(3):
                rows += [m2hi[..., d], m2lo[..., d], m2hi[..., d]]
            rows += [ones, ones, nhi, nlo]
        return np.stack(rows, axis=1)  # [B, K, N]

    return feats(p1, 1), feats(p2, 2)


def host_reduce(results) -> np.float32:
    """Combine per-core partials into the final scalar (fp64 on host)."""
    total = 0.0
    for r in results:
        d1 = np.asarray(r["d1out"], dtype=np.float64)  # [BPC, 128, NI]
        col = np.asarray(r["colout"]).astype(np.float64)  # [BPC, NJ, 128, MW]
        mean1 = d1.reshape(BPC, -1).mean(axis=1)  # [BPC]
        dist2 = col.min(axis=2)  # [BPC, NJ, MW]
        mean2 = dist2.reshape(BPC, -1).mean(axis=1)  # [BPC]
        total += float((mean1 + mean2).sum())
    return np.float32(total)


def make_in_maps(points1: np.ndarray, points2: np.ndarray):
    f1, f2 = make_features(points1, points2)
    in_maps = []
    for c in range(NCORES):
        sl = slice(c * BPC, (c + 1) * BPC)
        in_maps.append(
            {
                "feat1": np.ascontiguousarray(f1[sl]),
                "feat2": np.ascontiguousarray(f2[sl]),
            }
        )
    return in_maps


def kernel(points1: np.ndarray, points2: np.ndarray) -> np.ndarray:
    from concourse.bass_utils import run_bass_kernel_spmd

    nc = get_bass()
    in_maps = make_in_maps(points1, points2)
    res = run_bass_kernel_spmd(nc, in_maps, core_ids=list(range(NCORES)))
    return host_reduce(res.results)
